# revision 1
# baseline (speedup 1.0000x reference)
"""ConvGeodesic Trainium2 kernel.

conv[b,t,m,o] = sum_{q,c} w[b,m,t,q,c] * Y[b, idx[b,m,t,q,c], q, o]
where Y[b,v,q,o] = sum_n wsum[q,o,n] * signal[b,v,n], wsum = weights.sum((0,1)).
Then relu, L2-norm argmax over t, pick winning rotation, + bias.

Sharding: mesh-vertex axis m split over 8 cores (3750 rows each), fully local.
Each core computes the full Y table (2 x 30000 x 320 f32) into private HBM,
then dma_gathers 256-byte rows from it, weighted-sums on DVE, epilogue on ACT/DVE.
"""

import numpy as np
from contextlib import ExitStack

import concourse.bacc as bacc
import concourse.bass as bass
import concourse.mybir as mybir
import concourse.tile as tile
from concourse.tile_rust import add_dep_helper

F32 = mybir.dt.float32
I16 = mybir.dt.int16

B = 2
M = 30000
NCORES = 8
MC = M // NCORES          # 3750 rows per core
T = 8
Q = 5
NO = 64                   # feature dim
QON = Q * NO              # 320
NKT = 16                  # k*t weight slices
WROWS = NKT * Q * NO // 4 * 4  # 1280 weight rows (k,t,q,o)


def _cdiv(a, b):
    return (a + b - 1) // b


def build_program(mtot=M, mc=MC, gchunk=1, vgrp=5):
    """Build the SPMD per-core program. mc = real rows per core."""
    ng = _cdiv(mc, 128)              # g-blocks (padded)
    mp = ng * 128                    # padded rows per core
    assert ng % gchunk == 0
    nchunk = ng // gchunk            # m-chunks per batch
    nidx = gchunk * T * 128          # indices per gather call
    QCN = 15

    nc = bacc.Bacc("TRN2", target_bir_lowering=False, debug=False)

    sig = nc.dram_tensor("sig", [B, mtot, NO], F32, kind="ExternalInput")
    wts = nc.dram_tensor("wts", [WROWS, NO], F32, kind="ExternalInput")
    ident = nc.dram_tensor("ident", [128, 128], F32, kind="ExternalInput")
    baryp = nc.dram_tensor("baryp", [B, mp, 320], F32, kind="ExternalInput")
    biasp = nc.dram_tensor("biasp", [mp, NO], F32, kind="ExternalInput")
    outp = nc.dram_tensor("outp", [B, mc, NO], F32, kind="ExternalOutput")
    Yd = nc.dram_tensor("Yd", [B, mtot, QON], F32)

    def bcast(ap, n):
        """Append a 0-step dim of size n to an AP (free-dim broadcast)."""
        return ap.to_broadcast(list(ap.shape) + [n])

    MUL = mybir.AluOpType.mult

    with tile.TileContext(nc) as tc, ExitStack() as ctx:
        cpool = ctx.enter_context(tc.tile_pool(name="const", bufs=1))
        wldp = ctx.enter_context(tc.tile_pool(name="wld", bufs=1))
        tpsum = ctx.enter_context(tc.tile_pool(name="tpsum", bufs=2, space="PSUM"))
        ypsum = ctx.enter_context(tc.tile_pool(name="ypsum", bufs=2, space="PSUM"))
        sigp = ctx.enter_context(tc.tile_pool(name="sigp", bufs=2))
        stp = ctx.enter_context(tc.tile_pool(name="stp", bufs=2))
        ystg = ctx.enter_context(tc.tile_pool(name="ystg", bufs=1))
        bp = ctx.enter_context(tc.tile_pool(name="bary", bufs=1))
        w15p = ctx.enter_context(tc.tile_pool(name="w15", bufs=1))
        jp = ctx.enter_context(tc.tile_pool(name="jfold", bufs=1))
        i16p = ctx.enter_context(tc.tile_pool(name="i16p", bufs=1))
        idxp = ctx.enter_context(tc.tile_pool(name="idx", bufs=1))
        gp = ctx.enter_context(tc.tile_pool(name="gath", bufs=2))
        pp = ctx.enter_context(tc.tile_pool(name="prod", bufs=1))
        cvp = ctx.enter_context(tc.tile_pool(name="conv", bufs=1))
        actp = ctx.enter_context(tc.tile_pool(name="acts", bufs=2))
        nrmp = ctx.enter_context(tc.tile_pool(name="nrm", bufs=2))
        plp = ctx.enter_context(tc.tile_pool(name="pool", bufs=2))

        identS = cpool.tile([128, 128], F32)
        nc.sync.dma_start(identS[:], ident[:])

        biasT = cpool.tile([128, ng, NO], F32)
        nc.sync.dma_start(biasT[:], biasp[:].rearrange("(g p) n -> p g n", p=128))

        # ---- weights: transpose 1280x64 -> WT[64,1280], fold 16 kt-slices ----
        WT = cpool.tile([64, WROWS], F32)
        for j in range(WROWS // 128):
            wl = wldp.tile([128, NO], F32)
            nc.sync.dma_start(wl[:], wts[j * 128:(j + 1) * 128, :])
            wt_ps = tpsum.tile([64, 128], F32)
            nc.tensor.transpose(wt_ps[:], wl[:], identS[:])
            nc.scalar.copy(WT[:, j * 128:(j + 1) * 128], wt_ps[:])
        wsumT = cpool.tile([64, QON], F32)
        nc.vector.tensor_copy(wsumT[:], WT[:, 0:QON])
        for kt in range(1, NKT):
            nc.vector.tensor_add(wsumT[:], wsumT[:], WT[:, kt * QON:(kt + 1) * QON])

        # ---- Y table: per batch, per v-group of vgrp 128-blocks ----
        ydma = {b: [] for b in range(B)}
        nvg = _cdiv(mtot, vgrp * 128)
        for b in range(B):
            for vg in range(nvg):
                v0 = vg * vgrp * 128
                nv = min(vgrp * 128, mtot - v0)
                jfull = nv // 128
                rem = nv - jfull * 128
                SL = sigp.tile([128, vgrp, NO], F32)
                if jfull:
                    nc.sync.dma_start(
                        SL[:, :jfull, :],
                        sig[b, v0:v0 + jfull * 128, :].rearrange(
                            "(j p) n -> p j n", p=128))
                if rem:
                    nc.sync.dma_start(
                        SL[:rem, jfull, :], sig[b, v0 + jfull * 128:v0 + nv, :])
                Yst = ystg.tile([128, vgrp, QON], F32)
                nj = jfull + (1 if rem else 0)
                for j in range(nj):
                    pr = 128 if j < jfull else rem
                    st_ps = tpsum.tile([64, 128], F32, tag="st_ps")
                    nc.tensor.transpose(
                        st_ps[:, :pr], SL[:pr, j, :], identS[:pr, :pr])
                    ST = stp.tile([64, 128], F32)
                    nc.scalar.copy(ST[:, :pr], st_ps[:, :pr])
                    yp = ypsum.tile([128, QON], F32)
                    nc.tensor.matmul(
                        yp[:pr, :], ST[:, :pr], wsumT[:], start=True, stop=True)
                    nc.scalar.copy(Yst[:pr, j, :], yp[:pr, :])
                if jfull:
                    i1 = nc.sync.dma_start(
                        Yd[b, v0:v0 + jfull * 128, :].rearrange(
                            "(j p) n -> p j n", p=128), Yst[:, :jfull, :])
                    ydma[b].append(i1)
                if rem:
                    i2 = nc.sync.dma_start(
                        Yd[b, v0 + jfull * 128:v0 + nv, :], Yst[:rem, jfull, :])
                    ydma[b].append(i2)

        # ---- per batch: bary prep, idx build, gathers, compute ----
        Yv = Yd[:].rearrange("b v (q o) -> b v q o", q=Q)
        for b in range(B):
            B128 = bp.tile([128, ng, 320], F32)
            nc.sync.dma_start(
                B128[:], baryp[b].rearrange("(g p) s -> p g s", p=128))
            Br = B128[:].rearrange("p g (t q s) -> p s q t g", t=T, q=Q, s=8)
            W15 = w15p.tile([128, 3, Q, T, ng], F32)
            I128 = i16p.tile([128, 3, Q, T, ng], I16)
            for c in range(3):
                nc.vector.tensor_copy(W15[:, c], Br[:, 2 * c])
                nc.vector.tensor_copy(I128[:, c], Br[:, 2 * c + 1])

            # idx tile: [128, qc, g, t, ph] int16 (wrapped %16, replicated)
            idxb = idxp.tile([128, QCN, ng, T, 8], I16)
            i128f = I128[:].rearrange("p c q t g -> p (c q) t g")
            for ph in range(8):
                J = jp.tile([16, QCN, T, ng], I16)
                nc.sync.dma_start(J[:], i128f[16 * ph:16 * ph + 16])
                nc.vector.tensor_copy(
                    idxb[0:16, :, :, :, ph],
                    J[:].rearrange("p qc t g -> p qc g t"))
            nc.sync.dma_start(idxb[16:32], idxb[0:16])
            nc.sync.dma_start(idxb[32:64], idxb[0:32])
            nc.sync.dma_start(idxb[64:128], idxb[0:64])

            idxf = idxb[:].rearrange("p qc g t h -> p (qc g t h)")
            for ch in range(nchunk):
                g0 = ch * gchunk
                C = cvp.tile([128, gchunk, T, NO], F32)
                P = None
                for qc in range(QCN):
                    c, q = qc // Q, qc % Q
                    G = gp.tile([128, gchunk * T, NO], F32)
                    s0 = (qc * ng + g0) * T * 8
                    gi = nc.gpsimd.dma_gather(
                        G[:], Yv[b, :, q, :],
                        idxf[:, s0:s0 + nidx // 16],
                        nidx, nidx, NO, elem_step=QON)
                    for yi in ydma[b]:
                        add_dep_helper(gi.ins, yi.ins, reason="gather after Y")
                    Gv = G[:].rearrange("p (g t) n -> p g t n", t=T)
                    wv = bcast(
                        W15[:, c, q, :, g0:g0 + gchunk].rearrange(
                            "p t g -> p g t"), NO)
                    if qc == 0:
                        nc.vector.tensor_tensor(C[:], Gv, wv, op=MUL)
                    else:
                        P = pp.tile([128, gchunk, T, NO], F32, tag="prod")
                        nc.vector.tensor_tensor(P[:], Gv, wv, op=MUL)
                        nc.vector.tensor_add(C[:], C[:], P[:])

                # epilogue: relu, norms, argmax over t, pool, bias
                nc.scalar.activation(
                    C[:], C[:], mybir.ActivationFunctionType.Relu)
                nrm = nrmp.tile([128, gchunk, T], F32)
                sq = actp.tile([128, NO], F32, tag="sqscr")
                for g in range(gchunk):
                    for t in range(T):
                        nc.scalar.activation(
                            sq[:], C[:, g, t, :],
                            mybir.ActivationFunctionType.Square,
                            accum_out=nrm[:, g, t:t + 1])
                mx = nrmp.tile([128, gchunk], F32, tag="mx")
                nc.vector.tensor_reduce(
                    mx[:], nrm[:], axis=mybir.AxisListType.X,
                    op=mybir.AluOpType.max)
                msk = nrmp.tile([128, gchunk, T], F32, tag="msk")
                nc.vector.tensor_tensor(
                    msk[:], nrm[:], bcast(mx[:], T),
                    op=mybir.AluOpType.is_equal)
                M2 = pp.tile([128, gchunk, T, NO], F32, tag="prod")
                nc.vector.tensor_tensor(M2[:], C[:], bcast(msk[:], NO), op=MUL)
                pooled = plp.tile([128, gchunk, NO], F32)
                m2v = M2[:].rearrange("p g t n -> p g n t")
                nc.vector.tensor_reduce(
                    pooled[:], m2v, axis=mybir.AxisListType.X,
                    op=mybir.AluOpType.add)
                nc.vector.tensor_add(
                    pooled[:], pooled[:], biasT[:, g0:g0 + gchunk, :])

                # write out: rows m = (g0+g)*128 + p, clipped to mc
                r0 = g0 * 128
                rfull = min(mc - r0, gchunk * 128) // 128
                if rfull:
                    nc.sync.dma_start(
                        outp[b, r0:r0 + rfull * 128, :].rearrange(
                            "(g p) n -> p g n", p=128),
                        pooled[:, :rfull, :])
                rrem = min(mc - r0, gchunk * 128) - rfull * 128
                if rrem > 0:
                    nc.sync.dma_start(
                        outp[b, r0 + rfull * 128:r0 + rfull * 128 + rrem, :],
                        pooled[:rrem, rfull, :])
    return nc


_CACHE = {}


def _get_program(key=(M, MC, 1, 5)):
    if key not in _CACHE:
        nc = build_program(*key)
        nc.compile()
        _CACHE[key] = nc
    return _CACHE[key]


def _make_in_maps(signal, bary, weights, bias, mc=MC, ncores=NCORES):
    ng = _cdiv(mc, 128)
    mp = ng * 128
    signal = np.ascontiguousarray(signal, np.float32)
    wts = np.ascontiguousarray(weights, np.float32).reshape(WROWS, NO)
    ident = np.eye(128, dtype=np.float32)
    in_maps = []
    for cid in range(ncores):
        m0 = cid * mc
        bp = np.zeros((B, mp, 320), np.float32)
        bp[:, :mc] = bary[:, m0:m0 + mc].reshape(B, mc, 320)
        bi = np.zeros((mp, NO), np.float32)
        bi[:mc] = bias[m0:m0 + mc]
        in_maps.append(dict(sig=signal, wts=wts, ident=ident,
                            baryp=bp, biasp=bi))
    return in_maps


def kernel(signal, bary, weights, bias):
    from concourse.bass_utils import run_bass_kernel_spmd
    nc = _get_program()
    in_maps = _make_in_maps(np.asarray(signal, np.float32),
                            np.asarray(bary, np.float32),
                            np.asarray(weights, np.float32),
                            np.asarray(bias, np.float32))
    res = run_bass_kernel_spmd(nc, in_maps, core_ids=list(range(NCORES)))
    out = np.concatenate([res.results[c]["outp"] for c in range(NCORES)],
                         axis=1)
    return out.astype(np.float32)

